# revision 2
# baseline (speedup 1.0000x reference)
"""GNN message passing on 8 trn2 cores: edge-gather + one-hot matmul.

vs baseline:
  - dma_gather calls (<=4096 idxs) round-robin over 4 SWDGE queues so all
    8 Q7 cores generate descriptors concurrently.
  - S matrices (one-hot edge->dst) host-precomputed bf16, streamed per piece.
  - 13 small pieces with ring-3 buffers to avoid pipeline drains at piece
    boundaries; f32->bf16 cast on scalar, PSUM evacuation on vector.
"""

import numpy as np
import ml_dtypes

from concourse import bass, library_config, mybir
from concourse.bass_utils import run_bass_kernel_spmd

N_NODES = 50000
D = 64
N_CORES = 8
NODES_PER_CORE = N_NODES // N_CORES  # 6250
P = 128
N_TILES = (NODES_PER_CORE + P - 1) // P  # 49
TILES_PER_PIECE = 4
N_PIECES = (N_TILES + TILES_PER_PIECE - 1) // TILES_PER_PIECE  # 13
HALF_SPLIT = 32768
PSUM_BANKS = 8
MAX_GATHER_IDXS = 4096
N_QUEUES = 4
RING = 3

_f32 = mybir.dt.float32
_i16 = mybir.dt.int16
_bf16 = mybir.dt.bfloat16


def _round_up(a, b):
    return (a + b - 1) // b * b


def prepare(x, edge_index):
    dst = np.asarray(edge_index[0], dtype=np.int64)
    src = np.asarray(edge_index[1], dtype=np.int64)

    core = dst // NODES_PER_CORE
    dst_in_core = (dst - core * NODES_PER_CORE).astype(np.int32)
    tile = dst_in_core // P
    m = (dst_in_core % P).astype(np.int32)
    half = (src >= HALF_SPLIT).astype(np.int32)
    idx16 = np.where(half == 1, src - HALF_SPLIT, src).astype(np.int16)

    n_groups = N_TILES * 2
    counts = np.zeros((N_CORES, n_groups), dtype=np.int64)
    per_core = []
    for k in range(N_CORES):
        sel = np.nonzero(core == k)[0]
        g = (tile[sel] * 2 + half[sel]).astype(np.int64)
        order = np.argsort(g, kind="stable")
        sel = sel[order]
        g = g[order]
        # unique (group, src) count per group
        iv = idx16[sel].astype(np.int64)
        key = g * 65536 + iv
        counts[k] = np.bincount(
            g[np.unique(key, return_index=True)[1]], minlength=n_groups
        )
        per_core.append((sel, g))

    Vv = np.maximum(counts.max(axis=0), 1).astype(np.int64)
    V = _round_up(Vv, P).astype(np.int64)

    group_order = []
    for p in range(N_PIECES):
        tiles = range(p * TILES_PER_PIECE, min((p + 1) * TILES_PER_PIECE, N_TILES))
        for h in (0, 1):
            for t in tiles:
                group_order.append(t * 2 + h)
    group_order = np.array(group_order, dtype=np.int64)

    stream_off = np.zeros(n_groups, dtype=np.int64)
    off = 0
    for g in group_order:
        stream_off[g] = off
        off += V[g]
    total_v = off

    idx_cols = total_v // 16
    n_chunks = total_v // P

    idx_maps = []
    s_maps = []
    for k in range(N_CORES):
        sel, g = per_core[k]
        # dedup (group, src): one gather slot per unique src in a group
        iv = idx16[sel].astype(np.int64)
        order2 = np.lexsort((iv, g))
        sel = sel[order2]
        g = g[order2]
        iv = iv[order2]
        new_u = np.ones(len(sel), dtype=bool)
        new_u[1:] = (g[1:] != g[:-1]) | (iv[1:] != iv[:-1])
        uid = np.cumsum(new_u) - 1  # unique slot id (global ascending)
        gstart_u = np.searchsorted(uid[new_u], 0)  # dummy
        # rank of unique within its group
        u_g = g[new_u]
        u_iv = iv[new_u]
        grp_first = np.ones(len(u_g), dtype=bool)
        grp_first[1:] = u_g[1:] != u_g[:-1]
        grp_first_idx = np.maximum.accumulate(
            np.where(grp_first, np.arange(len(u_g)), 0)
        )
        u_rank = np.arange(len(u_g)) - grp_first_idx
        pos_u = stream_off[u_g] + u_rank
        pos_edge = pos_u[uid]

        idx_flat = np.zeros(total_v, dtype=np.int16)
        idx_flat[pos_u] = u_iv.astype(np.int16)
        idx_wrapped = np.ascontiguousarray(
            np.tile(idx_flat.reshape(-1, 16).T, (8, 1))
        )
        smat = np.zeros((n_chunks, P, P), dtype=np.float32)
        np.add.at(smat, (pos_edge // P, pos_edge % P, m[sel]), 1.0)
        s_cols = np.ascontiguousarray(
            smat.transpose(1, 0, 2).reshape(P, n_chunks * P)
        ).astype(ml_dtypes.bfloat16)
        idx_maps.append(idx_wrapped)
        s_maps.append(s_cols)

    meta = dict(
        V=V,
        total_v=int(total_v),
        idx_cols=int(idx_cols),
        n_chunks=int(n_chunks),
    )
    return idx_maps, s_maps, meta


def build_program(meta):
    V = meta["V"]
    idx_cols = meta["idx_cols"]
    n_chunks = meta["n_chunks"]

    chunks = []
    calls = []  # (piece, half, flat_off, cap, msgs_chunk_off)
    flat_off = 0
    for p in range(N_PIECES):
        tiles = list(
            range(p * TILES_PER_PIECE, min((p + 1) * TILES_PER_PIECE, N_TILES))
        )
        piece_local = 0
        for h in (0, 1):
            half_cap = int(sum(V[t * 2 + h] for t in tiles))
            done = 0
            while done < half_cap:
                sub = min(half_cap - done, MAX_GATHER_IDXS)
                calls.append((p, h, flat_off + done, sub, piece_local + done // P))
                done += sub
            for t in tiles:
                ng = int(V[t * 2 + h]) // P
                for j in range(ng):
                    chunks.append([p, piece_local, t, False, False])
                    piece_local += 1
            flat_off += half_cap

    first_seen = {}
    last_seen = {}
    for ci, (pp, lc, t, _, _) in enumerate(chunks):
        if t not in first_seen:
            first_seen[t] = ci
        last_seen[t] = ci
    for t, ci in first_seen.items():
        chunks[ci][3] = True
    for t, ci in last_seen.items():
        chunks[ci][4] = True
    assert len(chunks) == n_chunks

    piece_chunk_off = [0]
    acc = 0
    for p in range(N_PIECES):
        acc += sum(1 for c in chunks if c[0] == p)
        piece_chunk_off.append(acc)
    max_piece_chunks = max(
        sum(1 for c in chunks if c[0] == p) for p in range(N_PIECES)
    )
    calls_through = [0] * (N_PIECES + 1)
    for p in range(N_PIECES):
        calls_through[p + 1] = calls_through[p] + sum(1 for c in calls if c[0] == p)
    # cumulative calls among pieces sharing ring slot (p % RING)
    ring_calls_through = [0] * N_PIECES
    for p in range(N_PIECES):
        ring_calls_through[p] = sum(
            1 for c in calls if c[0] <= p and c[0] % RING == p % RING
        )
    # greedy size-balanced queue assignment (in issue order)
    qload = [0] * N_QUEUES
    call_queue = []
    for (p, h, foff, cap, mco) in calls:
        qi = min(range(N_QUEUES), key=lambda q: qload[q])
        call_queue.append(qi)
        qload[qi] += cap
    # per-piece flat offset and idx cols for per-piece idx loads
    piece_flat_off = [0] * (N_PIECES + 1)
    for p in range(N_PIECES):
        piece_flat_off[p + 1] = piece_flat_off[p] + sum(
            c[3] for c in calls if c[0] == p
        )
    max_piece_icols = max(
        (piece_flat_off[p + 1] - piece_flat_off[p]) // 16 for p in range(N_PIECES)
    )
    tiles_through = [0] * (N_PIECES + 1)
    for p in range(N_PIECES):
        nt = min((p + 1) * TILES_PER_PIECE, N_TILES) - p * TILES_PER_PIECE
        tiles_through[p + 1] = tiles_through[p] + nt

    tile_last_chunk = last_seen
    piece_first_chunk = {}
    for ci, (pp, lc, t, _, _) in enumerate(chunks):
        if pp not in piece_first_chunk:
            piece_first_chunk[pp] = ci

    nc = bass.Bass(num_swdge_queues=N_QUEUES)
    x = nc.declare_dram_parameter("x", [N_NODES, D], _f32, isOutput=False)
    idx = nc.declare_dram_parameter("idx", [P, idx_cols], _i16, isOutput=False)
    smat = nc.declare_dram_parameter("smat", [P, n_chunks * P], _bf16, isOutput=False)
    y = nc.declare_dram_parameter("y", [N_TILES * P, D], _f32, isOutput=True)

    import contextlib

    ctx = contextlib.ExitStack()
    idx_sb = [
        ctx.enter_context(nc.sbuf_tensor(f"idx{b}", [P, max_piece_icols], _i16))
        for b in range(RING)
    ]
    acc_sb = ctx.enter_context(nc.sbuf_tensor("acc_sb", [P, N_TILES * D], _f32))
    msgs_sb = [
        ctx.enter_context(
            nc.sbuf_tensor(f"msgs{b}", [P, max_piece_chunks * D], _f32)
        )
        for b in range(RING)
    ]
    msgsb_sb = [
        ctx.enter_context(
            nc.sbuf_tensor(f"msgsb{b}", [P, max_piece_chunks * D], _bf16)
        )
        for b in range(RING)
    ]
    s_sb = [
        ctx.enter_context(
            nc.sbuf_tensor(f"s{b}", [P, max_piece_chunks * P], _bf16)
        )
        for b in range(RING)
    ]
    psum = [
        ctx.enter_context(nc.psum_tensor(f"ps{i}", [P, D], _f32))
        for i in range(PSUM_BANKS)
    ]

    with (
        nc.Block() as block,
        nc.semaphore("ld_sem") as ld_sem,
        nc.semaphore("gq0") as gq0,
        nc.semaphore("gq1") as gq1,
        nc.semaphore("gq2") as gq2,
        nc.semaphore("sld_sem") as sld_sem,
        nc.semaphore("mm_sem") as mm_sem,
        nc.semaphore("cp_sem") as cp_sem,
        nc.semaphore("cast_sem") as cast_sem,
        nc.semaphore("st_sem") as st_sem,
    ):

        @block.sync
        def _(sync: bass.BassEngine):
            gqs = None  # set below (closure over gq sems)
            for p in range(N_PIECES):
                icols = (piece_flat_off[p + 1] - piece_flat_off[p]) // 16
                if p >= RING:
                    # idx ring slot free when gathers of piece p-RING are done
                    sync.wait_ge(
                        [gq0, gq1, gq2][(p - RING) % RING],
                        16 * ring_calls_through[p - RING],
                    )
                sync.dma_start(
                    out=idx_sb[p % RING][:, :icols],
                    in_=idx[
                        :,
                        piece_flat_off[p] // 16 : piece_flat_off[p + 1] // 16,
                    ],
                ).then_inc(ld_sem, 16)
                if p >= RING:
                    sync.wait_ge(mm_sem, piece_chunk_off[p - RING + 1])
                npc = piece_chunk_off[p + 1] - piece_chunk_off[p]
                sync.dma_start(
                    out=s_sb[p % RING][:, : npc * P],
                    in_=smat[
                        :, piece_chunk_off[p] * P : piece_chunk_off[p + 1] * P
                    ],
                ).then_inc(sld_sem, 16)
            sync.wait_ge(st_sem, 16 * N_PIECES)

        @block.gpsimd
        def _(gpsimd: bass.BassEngine):
            gpsimd.load_library(library_config.mlp)
            prev_piece = -1
            for call_i, (p, h, foff, cap, msgs_chunk_off) in enumerate(calls):
                if p != prev_piece:
                    gpsimd.wait_ge(ld_sem, 16 * (p + 1))
                    if p >= RING:
                        # msgs[p%RING] free when cast of piece p-RING done
                        gpsimd.wait_ge(cast_sem, p - RING + 1)
                prev_piece = p
                if h == 0:
                    src_view = x[0:HALF_SPLIT, :]
                else:
                    src_view = x[HALF_SPLIT:N_NODES, :]
                n_call_chunks = cap // P
                out_view = msgs_sb[p % RING][
                    :,
                    msgs_chunk_off * D : (msgs_chunk_off + n_call_chunks) * D,
                ].rearrange("p (c f) -> p c f", f=D)
                lo = foff - piece_flat_off[p]
                gpsimd.dma_gather(
                    out_ap=out_view,
                    in_ap=src_view,
                    idxs_ap=idx_sb[p % RING][:, lo // 16 : (lo + cap) // 16],
                    num_idxs=cap,
                    num_idxs_reg=cap,
                    elem_size=D,
                    single_packet=False,
                    queue_num=call_queue[call_i],
                ).then_inc([gq0, gq1, gq2][p % RING], 16)

        @block.scalar
        def _(scalar: bass.BassEngine):
            def store_piece(q):
                scalar.wait_ge(cp_sem, tiles_through[q + 1])
                r0 = q * TILES_PER_PIECE * P
                nt = tiles_through[q + 1] - tiles_through[q]
                scalar.dma_start(
                    out=y[r0 : r0 + nt * P].rearrange("(t p) f -> p t f", p=P),
                    in_=acc_sb[
                        :, tiles_through[q] * D : tiles_through[q + 1] * D
                    ].rearrange("p (t f) -> p t f", f=D),
                ).then_inc(st_sem, 16)

            for p in range(N_PIECES):
                scalar.wait_ge([gq0, gq1, gq2][p % RING], 16 * ring_calls_through[p])
                if p >= RING:
                    # msgsb[p%RING] free when matmuls of piece p-RING done
                    scalar.wait_ge(mm_sem, piece_chunk_off[p - RING + 1])
                npc = piece_chunk_off[p + 1] - piece_chunk_off[p]
                scalar.copy(
                    out=msgsb_sb[p % RING][:, : npc * D],
                    in_=msgs_sb[p % RING][:, : npc * D],
                ).then_inc(cast_sem, 1)
                if p >= 2:
                    store_piece(p - 2)
            store_piece(N_PIECES - 2)
            store_piece(N_PIECES - 1)

        @block.tensor
        def _(tensor: bass.BassEngine):
            for ci, (p, lc, t, start, stop) in enumerate(chunks):
                if ci == piece_first_chunk[p]:
                    tensor.wait_ge(sld_sem, 16 * (p + 1))
                    tensor.wait_ge(cast_sem, p + 1)
                if start and t >= PSUM_BANKS:
                    tensor.wait_ge(cp_sem, t - PSUM_BANKS + 1)
                tensor.matmul(
                    out=psum[t % PSUM_BANKS][:],
                    lhsT=s_sb[p % RING][:, lc * P : (lc + 1) * P],
                    rhs=msgsb_sb[p % RING][:, lc * D : (lc + 1) * D],
                    start=start,
                    stop=stop,
                    skip_group_check=True,
                ).then_inc(mm_sem, 1)

        @block.vector
        def _(vector: bass.BassEngine):
            for t in range(N_TILES):
                vector.wait_ge(mm_sem, tile_last_chunk[t] + 1)
                vector.tensor_copy(
                    out=acc_sb[:, t * D : (t + 1) * D],
                    in_=psum[t % PSUM_BANKS][:],
                ).then_inc(cp_sem, 1)

    ctx.close()
    from concourse.library_overlay import lower_extended_insts

    lower_extended_insts(nc)
    return nc


def kernel(x, edge_index):
    x = np.ascontiguousarray(np.asarray(x, dtype=np.float32))
    edge_index = np.asarray(edge_index)
    assert x.shape == (N_NODES, D)
    assert edge_index.shape[0] == 2

    idx_maps, s_maps, meta = prepare(x, edge_index)
    nc = build_program(meta)

    in_maps = [
        {"x": x, "idx": idx_maps[k], "smat": s_maps[k]} for k in range(N_CORES)
    ]
    import os

    trace = bool(int(os.environ.get("KERNEL_TRACE", "0")))
    res = run_bass_kernel_spmd(nc, in_maps, list(range(N_CORES)), trace=trace)
    if trace:
        kernel.last_results = res

    out = np.empty((N_NODES, D), dtype=np.float32)
    for k in range(N_CORES):
        out[k * NODES_PER_CORE : (k + 1) * NODES_PER_CORE] = res.results[k]["y"][
            :NODES_PER_CORE
        ]
    return out
